# revision 42
# baseline (speedup 1.0000x reference)
"""AFM attention layer Trainium2 kernel.

Math: reference computes
    scores[b,i,j,h] = sum_d x[b,i,d] x[b,j,d] w[h,d] + b[h]
    s = sum_h scores ; denom[b] = sum_ij s ; out = s / denom
The head sum collapses: with wsum[d] = sum_h w[h,d], bsum = sum_h b[h]:
    S[b] = (x[b] * wsum) @ x[b]^T          (64x64, symmetric)
    denom[b] = sum_ij S[b] + F^2 * bsum
    out[b] = (S[b] + bsum) / denom[b]
Sharding: data-parallel over batch, 512 samples per core on 8 cores.

Per-core pipeline (fp32 throughout), samples processed in pairs stacked on
the 128 SBUF partitions; engines balanced PE/ACT/GpSimd/DVE:
  1. DMA macro-tile of 32 samples as [128 (2 samples x 64 rows), 16*128]
     (SP HWDGE ring; first tile split x4 to cut the startup stall).
  2. PE transpose each pair block -> PSUM [128(d), 128(2x64 rows)].
  3. ScalarE copies PSUM -> xT (SBUF); GpSimd derives xwT = xT * wsum[p]
     (SBUF->SBUF tensor_scalar, keeps DVE free).
  4. Per sample: matmul lhsT=xwT[128,64], rhs=xT[128,64] -> S in PSUM;
     the two samples of a pair run col-tiled at tile_position (0,0)/(0,64)
     so they can execute concurrently on separate PE column groups.
  5. denom per 8-pair S block: DVE segmented tensor_reduce gives per-row
     sums [128, 8]; a half-mask PE matmul both finishes the partition
     reduction and replicates each sample's scalar across the right
     partition half; DVE adds F^2*bsum and takes the reciprocal. (This
     matches the reference's denominator algorithm, and avoids 480 extra
     full-width LDWEIGHTS a colsum-matmul scheme would cost.)
  6. (S + bsum) * inv in one DVE scalar_tensor_tensor per partition-half,
     inv broadcast along free dim with a stride-0 AP.
  7. DMA out per 8-pair block on the ACT HWDGE ring (overlaps input ring).
Built on bacc.Bacc: its compile pipeline splits multi-semaphore waits and
moves matmul waits onto LDWEIGHTS, which raw Bass+Tile output violates.
"""

import numpy as np

B, F, D = 4096, 64, 128
NCORES = 8
BS = B // NCORES            # 512 samples per core
MT_SAMPLES = 32             # samples per macro-tile
N_MT = BS // MT_SAMPLES     # 16 macro-tiles
G = MT_SAMPLES // 2         # 16 pairs per macro-tile

_CACHE = {}


DEFAULT_CFG = dict(tp=3, cp=1, sp=3, dp=2, x2=3, xt=6, xw=6, osb=4,
                   out_per_mt=False, denom_early=True, sw_pipe=False,
                   denom_from_s=True, inline_finish=True,
                   split_copies=False)


def _build(cfg: dict | None = None):
    import concourse.bass as bass  # noqa: F401
    import concourse.tile as tile
    from concourse import bacc, mybir

    cfg = {**DEFAULT_CFG, **(cfg or {})}
    fp32 = mybir.dt.float32
    AF = mybir.ActivationFunctionType

    nc = bacc.Bacc("TRN2", target_bir_lowering=False, debug=False,
                   num_devices=NCORES)

    x_in = nc.declare_dram_parameter("inputs", [BS, F, D], fp32,
                                     isOutput=False)
    cst_in = nc.declare_dram_parameter("consts", [128, 389], fp32,
                                       isOutput=False)
    out_d = nc.declare_dram_parameter("out", [BS, F, F], fp32, isOutput=True)

    with tile.TileContext(nc) as tc:
        with (
            tc.tile_pool(name="cst", bufs=1) as cstp,
            tc.tile_pool(name="x2", bufs=cfg["x2"]) as xp,
            tc.tile_pool(name="xt", bufs=cfg["xt"]) as xtp,
            tc.tile_pool(name="xw", bufs=cfg["xw"]) as xwp,
            tc.tile_pool(name="csq", bufs=2) as csqp,
            tc.tile_pool(name="db", bufs=2) as dbp,
            tc.tile_pool(name="inv", bufs=2) as invp,
            tc.tile_pool(name="osb", bufs=cfg["osb"]) as op,
            tc.tile_pool(name="tps", bufs=cfg["tp"], space="PSUM") as tp,
            tc.tile_pool(name="cps", bufs=cfg["cp"], space="PSUM") as cp,
            tc.tile_pool(name="sps", bufs=cfg["sp"], space="PSUM") as sp,
            tc.tile_pool(name="dps", bufs=cfg["dp"], space="PSUM") as dp,
        ):
            cst = cstp.tile([128, 389], fp32)
            cst_loaded = []

            def load_consts():
                # emitted after the first input quarter so the first
                # transpose's operands race down both HWDGE rings at once
                nc.scalar.dma_start(cst[:, 0:128], cst_in[:, 0:128])
                nc.scalar.dma_start(cst[:, 128:389], cst_in[:, 128:389])
                cst_loaded.append(True)

            ident = cst[:, 0:128]
            wsumB = cst[:, 128:256]
            wcol = cst[:, 256:257]
            mask2 = cst[:, 257:259]
            bsum_ap = cst[:, 259:260]
            f2b_ap = cst[:, 260:261]
            maskBC = cst[:, 261:389]

            def emit_denoms(C_ps):
                Csq = csqp.tile([128, MT_SAMPLES], fp32)
                nc.scalar.activation(Csq[:], C_ps[:], AF.Square)
                D_ps = dp.tile([128, MT_SAMPLES], fp32)
                nc.tensor.matmul(D_ps[:], wsumB, Csq[:],
                                 start=True, stop=True)
                Db = dbp.tile([128, MT_SAMPLES], fp32)
                nc.vector.tensor_scalar_add(Db[:], D_ps[:], f2b_ap)
                inv = invp.tile([128, MT_SAMPLES], fp32)
                nc.vector.reciprocal(inv[:], Db[:])
                return inv

            for mt in range(N_MT):
                X2 = xp.tile([128, G * 128], fp32)
                nsplit = 4 if mt == 0 else 1
                step = MT_SAMPLES // nsplit
                gstep = G // nsplit
                for sp_i in range(nsplit):
                    lo = mt * MT_SAMPLES + sp_i * step
                    src = x_in[lo:lo + step].rearrange(
                        "(g two) f d -> (two f) g d", two=2)
                    nc.sync.dma_start(
                        X2[:, sp_i * gstep * 128:(sp_i + 1) * gstep * 128]
                        .rearrange("p (g d) -> p g d", g=gstep), src)
                    if not cst_loaded:
                        load_consts()

                C_ps = None
                if not cfg["denom_from_s"]:
                    C_ps = cp.tile([128, MT_SAMPLES], fp32)
                S_blocks = []
                inv = None
                pending = []
                finished = set()

                def finish_block(sb, S_ps):
                    # denom per sample = sum_ij S + F^2*bsum, computed from
                    # S rowsums + a half-mask PE matmul that both finishes
                    # the partition reduction and replicates each sample's
                    # scalar across the matching partition half.
                    out_sb = op.tile([128, 512], fp32, name="out_sb",
                                     tag="out_sb")
                    s0 = 16 * sb
                    rsum = dbp.tile([128, 8], fp32, name="rsum", tag="rsum")
                    nc.vector.tensor_reduce(
                        rsum[:],
                        S_ps[:].rearrange("p (g j) -> p g j", j=64),
                        mybir.AxisListType.X, mybir.AluOpType.add)
                    D2_ps = dp.tile([128, 8], fp32, name="D2_ps",
                                    tag="D2_ps")
                    nc.tensor.matmul(D2_ps[:], maskBC, rsum[:],
                                     start=True, stop=True)
                    Db2 = csqp.tile([128, 8], fp32, name="Db2", tag="Db2")
                    nc.vector.tensor_scalar_add(Db2[:], D2_ps[:], f2b_ap)
                    inv2 = invp.tile([128, 8], fp32, name="inv2",
                                     tag="inv2")
                    nc.vector.reciprocal(inv2[:], Db2[:])
                    inv_top = inv2[0:64, 0:8].broadcast_to([64, 8, 64])
                    inv_bot = inv2[64:128, 0:8].broadcast_to([64, 8, 64])
                    nc.vector.scalar_tensor_tensor(
                        out_sb[0:64, :].rearrange("p (g j) -> p g j", j=64),
                        S_ps[0:64, :].rearrange("p (g j) -> p g j", j=64),
                        bsum_ap[0:64], inv_top,
                        mybir.AluOpType.add, mybir.AluOpType.mult)
                    nc.vector.scalar_tensor_tensor(
                        out_sb[64:128, :].rearrange("p (g j) -> p g j",
                                                    j=64),
                        S_ps[64:128, :].rearrange("p (g j) -> p g j", j=64),
                        bsum_ap[64:128], inv_bot,
                        mybir.AluOpType.add, mybir.AluOpType.mult)
                    dst = out_d[mt * MT_SAMPLES + s0:
                                mt * MT_SAMPLES + s0 + 16].rearrange(
                        "(g two) f j -> (two f) g j", two=2)
                    nc.scalar.dma_start(
                        dst, out_sb[:].rearrange("p (g j) -> p g j", g=8))
                    finished.add(sb)

                def emit_smms(tbq, xTq, xwTq):
                    if tbq % 2 == 0:
                        S_blocks.append(sp.tile([128, 512], fp32,
                                                name="S_ps", tag="S_ps"))
                    S_ps = S_blocks[-1]
                    for k in range(4):
                        g = tbq * 4 + k
                        c = (g % 8) * 64
                        lo = 128 * k
                        nc.tensor.matmul(
                            S_ps[0:64, c:c + 64],
                            xwTq[:, lo:lo + 64], xTq[:, lo:lo + 64],
                            start=True, stop=True, tile_position=(0, 0))
                        nc.tensor.matmul(
                            S_ps[64:128, c:c + 64],
                            xwTq[:, lo + 64:lo + 128],
                            xTq[:, lo + 64:lo + 128],
                            start=True, stop=True, tile_position=(0, 64))
                    if (tbq % 2 == 1 and cfg["denom_from_s"]
                            and not cfg["out_per_mt"]
                            and cfg["inline_finish"]
                            and mt == N_MT - 1):
                        finish_block(tbq // 2, S_ps)

                for tb in range(G // 4):
                    T_ps = tp.tile([128, 512], fp32)
                    for k in range(4):
                        g = tb * 4 + k
                        xblk = X2[:, 128 * g:128 * (g + 1)]
                        nc.tensor.transpose(
                            T_ps[:, 128 * k:128 * (k + 1)], xblk, ident)
                        if not cfg["denom_from_s"]:
                            nc.tensor.matmul(
                                C_ps[:, 2 * g:2 * g + 2], xblk, mask2,
                                start=True, stop=True)
                    if (tb == G // 4 - 1 and cfg["denom_early"]
                            and not cfg["denom_from_s"]):
                        inv = emit_denoms(C_ps)
                    xT = xtp.tile([128, 512], fp32)
                    xwT = xwp.tile([128, 512], fp32)
                    if cfg["split_copies"]:
                        for h in (0, 256):
                            nc.scalar.activation(xT[:, h:h + 256],
                                                 T_ps[:, h:h + 256], AF.Copy)
                            nc.gpsimd.tensor_scalar_mul(
                                xwT[:, h:h + 256], xT[:, h:h + 256], wcol)
                    else:
                        nc.scalar.activation(xT[:], T_ps[:], AF.Copy)
                        nc.gpsimd.tensor_scalar_mul(xwT[:], xT[:], wcol)
                    pending.append((tb, xT, xwT))
                    if not cfg["sw_pipe"] or len(pending) > 1:
                        emit_smms(*pending.pop(0))
                for item in pending:
                    emit_smms(*item)

                if inv is None and not cfg["denom_from_s"]:
                    inv = emit_denoms(C_ps)

                if cfg["denom_from_s"]:
                    for sb, S_ps in enumerate(S_blocks):
                        if sb not in finished:
                            finish_block(sb, S_ps)
                else:
                    for sb, S_ps in enumerate(S_blocks):
                        out_sb = op.tile([128, 512], fp32, name="out_sb",
                                         tag="out_sb")
                        s0 = 16 * sb
                        inv_top = inv[0:64, s0:s0 + 16:2].broadcast_to(
                            [64, 8, 64])
                        inv_bot = inv[64:128,
                                      s0 + 1:s0 + 16:2].broadcast_to(
                            [64, 8, 64])
                        nc.vector.scalar_tensor_tensor(
                            out_sb[0:64, :].rearrange(
                                "p (g j) -> p g j", j=64),
                            S_ps[0:64, :].rearrange(
                                "p (g j) -> p g j", j=64),
                            bsum_ap[0:64], inv_top,
                            mybir.AluOpType.add, mybir.AluOpType.mult)
                        nc.vector.scalar_tensor_tensor(
                            out_sb[64:128, :].rearrange(
                                "p (g j) -> p g j", j=64),
                            S_ps[64:128, :].rearrange(
                                "p (g j) -> p g j", j=64),
                            bsum_ap[64:128], inv_bot,
                            mybir.AluOpType.add, mybir.AluOpType.mult)
                        dst = out_d[mt * MT_SAMPLES + s0:
                                    mt * MT_SAMPLES + s0 + 16].rearrange(
                            "(g two) f j -> (two f) g j", two=2)
                        nc.scalar.dma_start(
                            dst,
                            out_sb[:].rearrange("p (g j) -> p g j", g=8))
    nc.finalize()
    return nc


def _consts_array(wsum: np.ndarray, bsum: float) -> np.ndarray:
    cst = np.zeros((128, 389), dtype=np.float32)
    cst[:, 0:128] = np.eye(128, dtype=np.float32)
    cst[:, 128:256] = wsum[:, None]          # wsum along contraction axis
    cst[:, 256] = wsum                       # per-partition scalar
    cst[0:64, 257] = 1.0                     # even-sample mask
    cst[64:128, 258] = 1.0                   # odd-sample mask
    cst[:, 259] = bsum
    cst[:, 260] = float(F * F) * bsum
    cst[0:64, 261:325] = 1.0                 # maskBC: top half -> even col blk
    cst[64:128, 325:389] = 1.0               # maskBC: bottom half -> odd blk
    return cst


def kernel(inputs: np.ndarray, w: np.ndarray, b: np.ndarray,
           trace: bool = False, tmpdir: str | None = None):
    from concourse.bass_utils import run_bass_kernel_spmd

    inputs = np.ascontiguousarray(np.asarray(inputs, dtype=np.float32))
    w = np.asarray(w, dtype=np.float32)
    b = np.asarray(b, dtype=np.float32)
    wsum = w.sum(axis=0)
    bsum = float(b.sum())

    if "nc" not in _CACHE:
        _CACHE["nc"] = _build()
    nc = _CACHE["nc"]

    cst = _consts_array(wsum, bsum)
    shards = inputs.reshape(NCORES, BS, F, D)
    in_maps = [{"inputs": shards[i], "consts": cst} for i in range(NCORES)]
    res = None
    for attempt in range(3):
        try:
            res = run_bass_kernel_spmd(nc, in_maps,
                                       core_ids=list(range(NCORES)),
                                       trace=trace, tmpdir=tmpdir)
            break
        except Exception:
            # transient device failures (NRT_EXEC_UNIT_UNRECOVERABLE) have
            # been observed sporadically on this fabric; retry.
            if attempt == 2:
                raise
    out = np.concatenate([r["out"] for r in res.results], axis=0)
    out = out.reshape(B, F, F).astype(np.float32)
    if trace:
        return out, res
    return out


if __name__ == "__main__":
    rng = np.random.default_rng(0)
    x = rng.standard_normal((B, F, D), dtype=np.float32)
    w = rng.standard_normal((4, D), dtype=np.float32)
    b = rng.standard_normal((4,), dtype=np.float32)
    out = kernel(x, w, b)
    wsum = w.sum(0)
    S = np.einsum('bid,bjd->bij', x * wsum, x) + b.sum()
    ref = S / S.sum(axis=(1, 2), keepdims=True)
    err = np.linalg.norm(out - ref) / np.linalg.norm(ref)
    print("rel err vs local ref:", err)
